# revision 1
# baseline (speedup 1.0000x reference)
"""Fused pre-LN multi-head attention (B=4, S=2048, D=1024, H=16) on 8 trn2 cores.

Sharding: core c -> batch b = c // 2, sequence-half = c % 2. Each core receives
ONLY its 1024-row half of the batch, runs LayerNorm + Q/K/V projections for
those rows, then exchanges its K^T / V halves with its partner core via
pairwise AllGathers so both cores of a batch hold full-sequence K/V. Attention
(16 heads) runs over the local 1024 query rows with keys ordered
[my half, partner half] (softmax is permutation-invariant in k). The output
projection produces the core's 1024 rows; the host concatenates.

Key scheduling structure:
  - K^T / V are exchanged in 512-col / 4-seq-tile chunks, issued as soon as
    each chunk's projection completes, so partner data arrives while local
    attention runs. Remote K/V land in their own tiles (KT_R0/1, V_R0/1) so
    Tile's dependency tracking cannot serialize local-half reads on the pulls.
  - Attention is kt-software-pipelined (scores kt+1 before ctx kt); the exp on
    the scalar engine is the intended pacer. The next pair's Q projection is
    hoisted into the current pair's tail so the PE never waits on the DVE at
    pair boundaries (keeps the HAM clock-gate warm).
  - Ctx chains (ones-column augmented: row 64 = sum(exp)) evacuate PSUM->SBUF
    immediately; softmax division happens one pair later, off the critical
    path. The reciprocal is spread across 16 partitions via a DMA round-trip
    (DVE reciprocal is ~8 cyc/elem/lane, so a [1,2048] row would cost ~16us).

LayerNorm gamma/beta and the 1/sqrt(head_dim) scale are folded into the
(host-pre-transposed, bf16) projection weights. Softmax skips max-subtraction
(scores are O(1) by construction).
"""

import numpy as np
import ml_dtypes

import concourse.bass as bass
import concourse.mybir as mybir
import concourse.tile as tile
from concourse import bacc
from concourse.bass import ds
from concourse.bass_utils import run_bass_kernel_spmd

F32 = mybir.dt.float32
BF16 = mybir.dt.bfloat16

B, S, D = 4, 2048, 1024
H, HD = 16, 64
EPS = 1e-6
P = 128
NDT = D // P          # 8  d-tiles
NST = S // P          # 16 seq tiles (full batch, both halves)
NST_L = NST // 2      # 8  local seq tiles
QROWS = S // 2        # 1024 rows per core
NQT = QROWS // P      # 8
NCORES = 8
HP = H // 2           # 8 head pairs
VSTRIDE = HD + 1      # 65: per-head V columns incl. the ones column


def build_program():
    nc = bacc.Bacc("TRN2", target_bir_lowering=False, enable_partition_id=True)

    x_d = nc.dram_tensor("x", [QROWS, D], F32, kind="ExternalInput")
    wqt_d = nc.dram_tensor("wqt", [D, D], BF16, kind="ExternalInput")
    wkt_d = nc.dram_tensor("wkt", [D, D], BF16, kind="ExternalInput")
    wvt_d = nc.dram_tensor("wvt", [D, D], BF16, kind="ExternalInput")
    wot_d = nc.dram_tensor("wot", [D, D], BF16, kind="ExternalInput")
    bq_d = nc.dram_tensor("bq", [NDT, P], F32, kind="ExternalInput")
    id_d = nc.dram_tensor("ident", [P, P], BF16, kind="ExternalInput")
    bo_d = nc.dram_tensor("bo", [1, D], F32, kind="ExternalInput")
    out_d = nc.dram_tensor("out", [QROWS, D], F32, kind="ExternalOutput")

    sub, mult, add = (
        mybir.AluOpType.subtract,
        mybir.AluOpType.mult,
        mybir.AluOpType.add,
    )
    AF = mybir.ActivationFunctionType
    GROUPS = [[0, 1], [2, 3], [4, 5], [6, 7]]

    with tile.TileContext(nc) as tc:
        with (
            tc.tile_pool(name="consts", bufs=1) as consts,
            tc.tile_pool(name="qt", bufs=1) as qt_pool,
            tc.tile_pool(name="kt", bufs=1) as kt_pool,
            tc.tile_pool(name="vp", bufs=1) as v_pool,
            tc.tile_pool(name="ctxt", bufs=1) as ct_pool,
            tc.tile_pool(name="xntp", bufs=1) as xnt_pool,
            tc.tile_pool(name="wq", bufs=1) as wq_pool,
            tc.tile_pool(name="dram", bufs=1, space="DRAM") as dram_pool,
        ):
            eps_t = consts.tile([P, 1], F32)
            nc.vector.memset(eps_t, EPS)
            bq_t = consts.tile([P, NDT], F32)
            nc.gpsimd.dma_start(out=bq_t, in_=bq_d.ap().rearrange("t p -> p t"))
            ident = consts.tile([P, P], BF16)
            nc.gpsimd.dma_start(out=ident, in_=id_d.ap())
            bob = consts.tile([P, D], F32)
            nc.sync.dma_start(out=bob, in_=bo_d.ap().to_broadcast([P, D]))

            # V layout per chunk: [p, seq_tile, head, 65]; v in cols 0:64,
            # ones column at 64 so the ctx matmul also produces the softmax
            # denominator (row 64). Local chunks are computed here; remote
            # chunks arrive via AllGather pulls (value cols only; ones are
            # memset locally).
            V_L = v_pool.tile([P, NST_L, H * VSTRIDE], BF16, name="V_L")
            V_R = v_pool.tile([P, NST_L, H * VSTRIDE], BF16, name="V_R")
            VrL = V_L.rearrange("p s (h e) -> p s h e", e=VSTRIDE)
            VrR = V_R.rearrange("p s (h e) -> p s h e", e=VSTRIDE)
            nc.vector.memset(VrL[:, :, :, HD : HD + 1], 1.0)
            nc.vector.memset(VrR[:, :, :, HD : HD + 1], 1.0)

            def v_tile(kt):
                # (Vr chunk, local index) for global k-tile kt
                if kt < NST_L:
                    return VrL, kt
                return VrR, kt - NST_L

            QT = qt_pool.tile([P, NDT, QROWS], BF16)
            KT_L = kt_pool.tile([P, NDT, QROWS], BF16, name="KT_L")
            KT_R = kt_pool.tile([P, NDT, QROWS], BF16, name="KT_R")
            CT = ct_pool.tile([P, NDT, QROWS], BF16)
            XNT = xnt_pool.tile([P, NDT, QROWS], BF16)
            WQ = wq_pool.tile([P, NDT, D], BF16)

            ibk = dram_pool.tile([P, NDT, QROWS], BF16, name="ibk", tag="ibk")
            obk = dram_pool.tile(
                [2, P, NDT, QROWS], BF16, name="obk", tag="obk"
            )
            ibv = dram_pool.tile([P, NST_L, H, HD], BF16, name="ibv", tag="ibv")
            obv = dram_pool.tile(
                [2, P, NST_L, H, HD], BF16, name="obv", tag="obv"
            )

            # DRAM bounce buffers for the reciprocal partition spread/gather
            recd_a = dram_pool.tile([1, 4 * 512], F32, name="recd_a", tag="recd_a")
            recd_b = dram_pool.tile([1, 16, 128], F32, name="recd_b", tag="recd_b")

            pid = nc.gpsimd.partition_id()
            sel = 1 - (pid % 2)

            # ---- phase 1: LN + K/V projections + exchanges ----------------
            with (
                tc.tile_pool(name="wk", bufs=1) as wk_pool,
                tc.tile_pool(name="wv", bufs=1) as wv_pool,
                tc.tile_pool(name="xp", bufs=3) as x_pool,
                tc.tile_pool(name="xnp", bufs=2) as xn_pool,
                tc.tile_pool(name="statp", bufs=6) as stat_pool,
                tc.tile_pool(name="psum_proj", bufs=2, space="PSUM") as psum_proj,
            ):
                WK = wk_pool.tile([P, NDT, D], BF16)
                WV = wv_pool.tile([P, NDT, D], BF16)

                def load_w(eng, W, w_d):
                    for _t in range(NDT):
                        eng.dma_start(
                            out=W[:, _t, :],
                            in_=w_d.ap().rearrange("(t p) j -> p t j", p=P)[
                                :, _t, :
                            ],
                        )

                x_eng = [nc.sync, nc.scalar, nc.gpsimd]

                def ln_tile(s):
                    xt = x_pool.tile([P, D], F32, name="xt", tag="x")
                    x_eng[s % 3].dma_start(
                        out=xt, in_=x_d.ap()[s * P : (s + 1) * P, :]
                    )
                    st = stat_pool.tile([P, 2, 6], F32, name="st", tag="st")
                    nc.vector.bn_stats(out=st[:, 0], in_=xt[:, 0:512])
                    nc.vector.bn_stats(out=st[:, 1], in_=xt[:, 512:1024])
                    mv = stat_pool.tile([P, 2], F32, name="mv", tag="mv")
                    nc.vector.bn_aggr(out=mv, in_=st)
                    std = stat_pool.tile([P, 1], F32, name="sd", tag="sd")
                    nc.scalar.activation(
                        out=std, in_=mv[:, 1:2], func=AF.Sqrt, bias=eps_t
                    )
                    rstd = stat_pool.tile([P, 1], F32, name="rs", tag="rs")
                    nc.vector.reciprocal(out=rstd, in_=std)
                    xn = xn_pool.tile([P, D], BF16, name="xn", tag="xn")
                    nc.vector.tensor_scalar(
                        out=xn,
                        in0=xt,
                        scalar1=mv[:, 0:1],
                        scalar2=rstd,
                        op0=sub,
                        op1=mult,
                    )
                    for db in range(NDT):
                        ptr = psum_proj.tile([P, P], BF16, name="ptr", tag="tr")
                        nc.tensor.transpose(
                            ptr, xn[:, db * P : (db + 1) * P], ident
                        )
                        nc.vector.tensor_copy(
                            XNT[:, db, s * P : (s + 1) * P], ptr
                        )

                def v_proj(s):
                    for df in range(2):
                        ps = psum_proj.tile([P, 512], F32, name="ps", tag="pp")
                        for k in range(NDT):
                            nc.tensor.matmul(
                                ps,
                                lhsT=XNT[:, k, s * P : (s + 1) * P],
                                rhs=WV[:, k, df * 512 : (df + 1) * 512],
                                start=(k == 0),
                                stop=(k == NDT - 1),
                            )
                        ps_h = ps.rearrange("p (h e) -> p h e", e=HD)
                        nc.scalar.activation(
                            out=VrL[:, s, df * 8 : (df + 1) * 8, 0:HD],
                            in_=ps_h,
                            func=AF.Copy,
                        )

                def k_proj(kf):
                    for i in range(NDT):
                        ps = psum_proj.tile([P, 512], F32, name="ps", tag="pp")
                        for k in range(NDT):
                            nc.tensor.matmul(
                                ps,
                                lhsT=WK[:, k, i * P : (i + 1) * P],
                                rhs=XNT[:, k, kf * 512 : (kf + 1) * 512],
                                start=(k == 0),
                                stop=(k == NDT - 1),
                            )
                        nc.scalar.activation(
                            out=KT_L[:, i, kf * 512 : (kf + 1) * 512],
                            in_=ps,
                            func=AF.Copy,
                        )

                def exch_k():
                    nc.sync.dma_start(out=ibk[:], in_=KT_L[:])
                    nc.gpsimd.collective_compute(
                        "AllGather",
                        mybir.AluOpType.bypass,
                        replica_groups=GROUPS,
                        ins=[ibk.opt()],
                        outs=[obk.opt()],
                    )
                    nc.gpsimd.dma_start(
                        out=KT_R[:], in_=obk[ds(sel, 1), :, :, :]
                    )

                def exch_v():
                    nc.sync.dma_start(out=ibv[:], in_=VrL[:, :, :, 0:HD])
                    nc.gpsimd.collective_compute(
                        "AllGather",
                        mybir.AluOpType.bypass,
                        replica_groups=GROUPS,
                        ins=[ibv.opt()],
                        outs=[obv.opt()],
                    )
                    nc.gpsimd.dma_start(
                        out=VrR[:, 0:4, :, 0:HD],
                        in_=obv[ds(sel, 1), :, 0:4, :, :],
                    )
                    nc.gpsimd.dma_start(
                        out=VrR[:, 4:8, :, 0:HD],
                        in_=obv[ds(sel, 1), :, 4:8, :, :],
                    )

                for s in range(4):
                    ln_tile(s)
                load_w(nc.gpsimd, WK, wkt_d)
                for s in range(4, 8):
                    ln_tile(s)
                load_w(nc.scalar, WV, wvt_d)
                k_proj(0)
                k_proj(1)
                exch_k()
                load_w(nc.gpsimd, WQ, wqt_d)
                for s in range(4):
                    v_proj(s)
                for s in range(4, 8):
                    v_proj(s)
                exch_v()
                # pair-0 Q projection here, so attention's first scores are
                # not gated by the attention-psum WAR on phase-1 banks
                for qf in range(2):
                    qp = psum_proj.tile([P, 512], F32, name="qp", tag="pp")
                    for k in range(NDT):
                        nc.tensor.matmul(
                            qp,
                            lhsT=WQ[:, k, 0:P],
                            rhs=XNT[:, k, qf * 512 : (qf + 1) * 512],
                            start=(k == 0),
                            stop=(k == NDT - 1),
                        )
                    nc.vector.tensor_scalar(
                        out=QT[:, 0, qf * 512 : (qf + 1) * 512],
                        in0=qp,
                        scalar1=bq_t[:, 0:1],
                        scalar2=None,
                        op0=add,
                    )

            # ---- phase 2: attention --------------------------------------
            with (
                tc.tile_pool(name="wo", bufs=1) as wo_pool,
                tc.tile_pool(name="crp", bufs=2) as cr_pool,
                tc.tile_pool(name="sep", bufs=2) as se_pool,
            ):
              WO = wo_pool.tile([P, NDT, D], BF16)
              for _t in range(NDT):
                  nc.sync.dma_start(
                      out=WO[:, _t, :],
                      in_=wot_d.ap().rearrange("(t p) j -> p t j", p=P)[:, _t, :],
                  )

              seb_cache = [None]

              def emit_normalize(ent, qfs=(0, 1), spread=True):
                  pt, cr = ent
                  if spread:
                      # spread the denominator row [1, 4*512] across 16
                      # partitions for the iterative-divide reciprocal
                      # (DVE reciprocal is ~8 cyc/elem/lane), via DRAM
                      nc.sync.dma_start(
                          out=recd_a[:],
                          in_=cr[HD : HD + 1, :, :].rearrange("p c q -> p (c q)"),
                      )
                      rs = se_pool.tile([16, 128], F32, name="rs", tag="rs")
                      nc.sync.dma_start(
                          out=rs,
                          in_=recd_a.rearrange("p (a b) -> (p a) b", a=16),
                      )
                      rr = se_pool.tile([16, 128], F32, name="rr", tag="rr")
                      nc.vector.reciprocal(out=rr, in_=rs)
                      nc.sync.dma_start(out=recd_b[0], in_=rr)
                      se0 = se_pool.tile(
                          [1, 4, 512], F32, name="se0", tag="se0", bufs=1
                      )
                      nc.sync.dma_start(
                          out=se0,
                          in_=recd_b.rearrange("p a b -> p (a b)").rearrange(
                              "p (c q) -> p c q", q=512
                          ),
                      )
                      seb_cache[0] = se0
                  se0 = seb_cache[0]
                  for qf in qfs:
                      for hi in range(2):
                          ch = hi * 2 + qf
                          seb = se_pool.tile([P, 512], F32, name="seb", tag="seb")
                          nc.gpsimd.partition_broadcast(seb[0:HD, :], se0[:, ch, :])
                          if hi == 0:
                              nc.vector.tensor_tensor(
                                  out=CT[0:HD, pt, qf * 512 : (qf + 1) * 512],
                                  in0=cr[0:HD, ch, :],
                                  in1=seb[0:HD, :],
                                  op=mult,
                              )
                          else:
                              tmp = se_pool.tile(
                                  [HD, 512], BF16, name="ctmp", tag="ctmp", bufs=1
                              )
                              nc.vector.tensor_tensor(
                                  out=tmp,
                                  in0=cr[0:HD, ch, :],
                                  in1=seb[0:HD, :],
                                  op=mult,
                              )
                              # partition shift 0..63 -> 64..127 via DMA
                              nc.gpsimd.dma_start(
                                  out=CT[HD:P, pt, qf * 512 : (qf + 1) * 512],
                                  in_=tmp,
                              )

              last_cr = [None]
              with (
                tc.tile_pool(name="probs", bufs=12) as probs_pool,
                tc.tile_pool(name="psum_sc", bufs=2, space="PSUM") as psum_sc,
                tc.tile_pool(name="psum_cx", bufs=4, space="PSUM") as psum_cx,
              ):
                def q_jit(tt):
                    qps = psum_sc.tile([P, QROWS], F32, name="qps", tag="s")
                    for qf in range(2):
                        for k in range(NDT):
                            nc.tensor.matmul(
                                qps[:, qf * 512 : (qf + 1) * 512],
                                lhsT=WQ[:, k, tt * P : (tt + 1) * P],
                                rhs=XNT[:, k, qf * 512 : (qf + 1) * 512],
                                start=(k == 0),
                                stop=(k == NDT - 1),
                            )
                    nc.vector.tensor_scalar(
                        out=QT[:, tt, :],
                        in0=qps,
                        scalar1=bq_t[:, tt : tt + 1],
                        scalar2=None,
                        op0=add,
                    )

                pending_norm = [None]
                for t in range(HP):
                    if pending_norm[0] is not None:
                        emit_normalize(pending_norm[0])
                        pending_norm[0] = None

                    probs = [[None] * NST for _ in range(2)]
                    cx = [[None] * 2 for _ in range(2)]  # [hi][qf]

                    def emit_scores(kt):
                        if kt < NST_L:
                            kl = KT_L[:, t, kt * P : (kt + 1) * P]
                        else:
                            i = kt - NST_L
                            kl = KT_R[:, t, i * P : (i + 1) * P]
                        for hi in range(2):
                            off = hi * HD
                            sps = psum_sc.tile([P, QROWS], F32, name="sps", tag="s")
                            for qf in range(2):
                                nc.tensor.matmul(
                                    sps[:, qf * 512 : (qf + 1) * 512],
                                    lhsT=kl[off : off + HD, :],
                                    rhs=QT[off : off + HD, t, qf * 512 : (qf + 1) * 512],
                                    start=True,
                                    stop=True,
                                    tile_position=(off, 0),
                                )
                            pt = probs_pool.tile([P, QROWS], BF16, name="pt", tag="p")
                            nc.scalar.activation(out=pt, in_=sps, func=AF.Exp)
                            probs[hi][kt] = pt

                    def emit_ctx(kt):
                        vr, vi = v_tile(kt)
                        for hi in range(2):
                            h = 2 * t + hi
                            for qf in range(2):
                                if kt == 0:
                                    cx[hi][qf] = psum_cx.tile(
                                        [P, 512], F32, name="cx", tag="cx"
                                    )
                                nc.tensor.matmul(
                                    cx[hi][qf][0:VSTRIDE, :],
                                    lhsT=vr[:, vi, h, :],
                                    rhs=probs[hi][kt][:, qf * 512 : (qf + 1) * 512],
                                    start=(kt == 0),
                                    stop=(kt == NST - 1),
                                )

                    for kt in range(NST):
                        emit_scores(kt)
                        if kt == NST - 2 and t + 1 < HP:
                            q_jit(t + 1)
                        if kt >= 1:
                            emit_ctx(kt - 1)
                    emit_ctx(NST - 1)

                    # evacuate ctx chains to SBUF fast so the PSUM banks free
                    # up for the next pair; rows 0..63 = unnormalized ctx,
                    # row 64 = sum(exp)
                    cr = cr_pool.tile([VSTRIDE, 4, 512], F32, name="cr", tag="cr")
                    for hi in range(2):
                        for qf in range(2):
                            nc.vector.tensor_copy(
                                cr[:, hi * 2 + qf, :], cx[hi][qf][0:VSTRIDE, :]
                            )
                    if t == HP - 1:
                        last_cr[0] = (t, cr)
                    else:
                        pending_norm[0] = (t, cr)

              # ---- final normalize + output projection, qf-interleaved ----
              with (
                  tc.tile_pool(name="osb", bufs=3) as osb_pool,
                  tc.tile_pool(name="psum_o", bufs=8, space="PSUM") as psum_o,
              ):
                  def out_proj_half(qts):
                      # accumulate pairs 0..6 for all chains first; the pair-7
                      # contribution lands after its normalize completes
                      chains = {}
                      for qt in qts:
                          for jf in range(2):
                              ps = psum_o.tile([P, 512], F32, name="ps", tag="po")
                              chains[(qt, jf)] = ps
                              for i in range(NDT - 1):
                                  nc.tensor.matmul(
                                      ps,
                                      lhsT=CT[:, i, qt * P : (qt + 1) * P],
                                      rhs=WO[:, i, jf * 512 : (jf + 1) * 512],
                                      start=(i == 0),
                                      stop=False,
                                  )
                      for qt in qts:
                          ot = osb_pool.tile([P, D], F32, name="ot", tag="o")
                          for jf in range(2):
                              ps = chains[(qt, jf)]
                              nc.tensor.matmul(
                                  ps,
                                  lhsT=CT[:, NDT - 1, qt * P : (qt + 1) * P],
                                  rhs=WO[:, NDT - 1, jf * 512 : (jf + 1) * 512],
                                  start=False,
                                  stop=True,
                              )
                              nc.vector.tensor_tensor(
                                  out=ot[:, jf * 512 : (jf + 1) * 512],
                                  in0=ps,
                                  in1=bob[:, jf * 512 : (jf + 1) * 512],
                                  op=add,
                              )
                          nc.sync.dma_start(
                              out=out_d.ap()[qt * P : (qt + 1) * P, :], in_=ot
                          )

                  emit_normalize(last_cr[0], qfs=(0,))
                  out_proj_half(range(4))
                  emit_normalize(last_cr[0], qfs=(1,), spread=False)
                  out_proj_half(range(4, NQT))

    nc.compile()
    return nc


_NC_CACHE = None


def _get_program():
    global _NC_CACHE
    if _NC_CACHE is None:
        _NC_CACHE = build_program()
    return _NC_CACHE


def _prep_host(x, ln_gamma, ln_beta, Wq, bq, Wk, bk, Wv, bv, Wo, bo):
    bf16 = ml_dtypes.bfloat16
    g = np.asarray(ln_gamma, np.float64)
    be = np.asarray(ln_beta, np.float64)
    scale = 1.0 / np.sqrt(np.float64(HD))

    def fold(W, b, s=1.0):
        W = np.asarray(W, np.float64)
        b = np.asarray(b, np.float64)
        W_eff = W * g[None, :] * s
        b_eff = (b + W @ be) * s
        wt = np.ascontiguousarray(W_eff.T).astype(bf16)
        return wt, b_eff.astype(np.float32)

    wqt, bq_e = fold(Wq, bq, scale)
    wkt, _ = fold(Wk, bk)           # K bias cancels in softmax
    wvt, bv_e = fold(Wv, bv)
    Wo64 = np.asarray(Wo, np.float64)
    wot = np.ascontiguousarray(Wo64.T).astype(bf16)
    # ctx rows carry +bv_eff (per-head value bias); fold it through Wo into bo
    bo_e = (np.asarray(bo, np.float64) + Wo64 @ np.asarray(bv_e, np.float64)
            ).astype(np.float32)

    shared = {
        "wqt": wqt,
        "wkt": wkt,
        "wvt": wvt,
        "wot": wot,
        "bq": bq_e.reshape(NDT, P),
        "bo": bo_e.reshape(1, D),
        "ident": np.eye(P, dtype=bf16),
    }
    x = np.asarray(x, np.float32)
    in_maps = []
    for c in range(NCORES):
        b_idx, half = c // 2, c % 2
        x_local = x[b_idx, half * QROWS : (half + 1) * QROWS]
        in_maps.append({"x": np.ascontiguousarray(x_local), **shared})
    return in_maps


def kernel(x, ln_gamma, ln_beta, Wq, bq, Wk, bk, Wv, bv, Wo, bo):
    nc = _get_program()
    in_maps = _prep_host(x, ln_gamma, ln_beta, Wq, bq, Wk, bk, Wv, bv, Wo, bo)
    res = run_bass_kernel_spmd(nc, in_maps, core_ids=list(range(NCORES)))
    out = np.empty((B, S, D), np.float32)
    for c in range(NCORES):
        b_idx, half = c // 2, c % 2
        out[b_idx, half * QROWS : (half + 1) * QROWS] = res.results[c]["out"]
    return out


if __name__ == "__main__":
    build_program()
    print("program built OK")



# revision 2
# speedup vs baseline: 1.0497x; 1.0497x over previous
"""Fused pre-LN multi-head attention (B=4, S=2048, D=1024, H=16) on 8 trn2 cores.

Sharding: core c -> batch b = c // 2, query-half = c % 2. Each core receives
the FULL batch-b sequence (2048 rows), host-reordered so the core's own
1024 query rows come first: [my half, partner half]. The core runs LayerNorm
and K/V projections for the whole sequence (duplicating its partner's K/V
work), so NO collectives are needed; softmax is permutation-invariant in k,
so the key order [my half, partner half] is harmless. Q/attention/output
projection cover only the local 1024 query rows; the host concatenates.

Key scheduling structure:
  - Attention is kt-software-pipelined (scores kt+1 before ctx kt); the exp on
    the scalar engine is the intended pacer. The next pair's Q projection is
    hoisted into the current pair's tail so the PE never waits on the DVE at
    pair boundaries (keeps the HAM clock-gate warm).
  - Scores for the two heads of a pair are issued at tile_position (0,0) and
    (64,0): distinct PE row-groups, so they can overlap on hardware.
  - Ctx chains (ones-column augmented: row 64 = sum(exp)) evacuate PSUM->SBUF
    immediately; softmax division happens one pair later, off the critical
    path. The reciprocal is spread across 16 partitions via a DMA round-trip
    (DVE reciprocal is ~8 cyc/elem/lane, so a [1,2048] row would cost ~16us).
  - K/V evacuations run on the DVE; the scalar engine is reserved for exp,
    which is the attention-phase bottleneck (~33.5M exps/core).

LayerNorm gamma/beta and the 1/sqrt(head_dim) scale are folded into the
(host-pre-transposed, bf16) projection weights. Softmax skips max-subtraction
(scores are O(1) by construction).
"""

import numpy as np
import ml_dtypes

import concourse.bass as bass
import concourse.mybir as mybir
import concourse.tile as tile
from concourse import bacc
from concourse.bass import ds
from concourse.bass_utils import run_bass_kernel_spmd

F32 = mybir.dt.float32
BF16 = mybir.dt.bfloat16

B, S, D = 4, 2048, 1024
H, HD = 16, 64
EPS = 1e-6
P = 128
NDT = D // P          # 8  d-tiles
NST = S // P          # 16 seq tiles (full batch sequence)
QROWS = S // 2        # 1024 local query rows per core
NQT = QROWS // P      # 8
NCORES = 8
HP = H // 2           # 8 head pairs
VSTRIDE = HD + 1      # 65: per-head V columns incl. the ones column


def build_program():
    nc = bacc.Bacc("TRN2", target_bir_lowering=False)

    x_d = nc.dram_tensor("x", [S, D], F32, kind="ExternalInput")
    wqt_d = nc.dram_tensor("wqt", [D, D], BF16, kind="ExternalInput")
    wkt_d = nc.dram_tensor("wkt", [D, D], BF16, kind="ExternalInput")
    wvt_d = nc.dram_tensor("wvt", [D, D], BF16, kind="ExternalInput")
    wot_d = nc.dram_tensor("wot", [D, D], BF16, kind="ExternalInput")
    bq_d = nc.dram_tensor("bq", [NDT, P], F32, kind="ExternalInput")
    id_d = nc.dram_tensor("ident", [P, P], BF16, kind="ExternalInput")
    bo_d = nc.dram_tensor("bo", [1, D], F32, kind="ExternalInput")
    out_d = nc.dram_tensor("out", [QROWS, D], F32, kind="ExternalOutput")

    sub, mult, add = (
        mybir.AluOpType.subtract,
        mybir.AluOpType.mult,
        mybir.AluOpType.add,
    )
    AF = mybir.ActivationFunctionType

    with tile.TileContext(nc) as tc:
        with (
            tc.tile_pool(name="consts", bufs=1) as consts,
            tc.tile_pool(name="qt", bufs=1) as qt_pool,
            tc.tile_pool(name="kt", bufs=1) as kt_pool,
            tc.tile_pool(name="vp", bufs=1) as v_pool,
            tc.tile_pool(name="ctxt", bufs=1) as ct_pool,
            tc.tile_pool(name="xntp", bufs=1) as xnt_pool,
            tc.tile_pool(name="wq", bufs=1) as wq_pool,
            tc.tile_pool(name="dram", bufs=1, space="DRAM") as dram_pool,
        ):
            eps_t = consts.tile([P, 1], F32)
            nc.vector.memset(eps_t, EPS)
            bq_t = consts.tile([P, NDT], F32)
            nc.gpsimd.dma_start(out=bq_t, in_=bq_d.ap().rearrange("t p -> p t"))
            ident = consts.tile([P, P], BF16)
            nc.gpsimd.dma_start(out=ident, in_=id_d.ap())
            bob = consts.tile([P, D], F32)
            nc.sync.dma_start(out=bob, in_=bo_d.ap().to_broadcast([P, D]))

            # V layout: [p, seq_tile, head, 65]; v in cols 0:64, ones column
            # at 64 so the ctx matmul also produces the softmax denominator
            # (row 64).
            V = v_pool.tile([P, NST, H * VSTRIDE], BF16, name="V")
            Vr = V.rearrange("p s (h e) -> p s h e", e=VSTRIDE)
            nc.vector.memset(Vr[:, :, :, HD : HD + 1], 1.0)

            QT = qt_pool.tile([P, NDT, QROWS], BF16)
            KT = kt_pool.tile([P, NDT, S], BF16, name="KT")
            CT = ct_pool.tile([P, NDT, QROWS], BF16)
            # local-half xn^T lives through attention (q_jit reads it)
            XNT = xnt_pool.tile([P, NDT, QROWS], BF16)
            WQ = wq_pool.tile([P, NDT, D], BF16)

            # DRAM bounce buffers for the reciprocal partition spread/gather
            recd_a = dram_pool.tile([1, 4 * 512], F32, name="recd_a", tag="recd_a")
            recd_b = dram_pool.tile([1, 16, 128], F32, name="recd_b", tag="recd_b")

            # ---- phase 1: LN + K/V projections (full sequence) ------------
            with (
                tc.tile_pool(name="wk", bufs=1) as wk_pool,
                tc.tile_pool(name="wv", bufs=1) as wv_pool,
                tc.tile_pool(name="xntr", bufs=1) as xntr_pool,
                tc.tile_pool(name="xp", bufs=3) as x_pool,
                tc.tile_pool(name="xnp", bufs=2) as xn_pool,
                tc.tile_pool(name="statp", bufs=6) as stat_pool,
                tc.tile_pool(name="psum_proj", bufs=2, space="PSUM") as psum_proj,
            ):
                WK = wk_pool.tile([P, NDT, D], BF16)
                WV = wv_pool.tile([P, NDT, D], BF16)
                # remote-half xn^T: only needed for K/V projections
                XNTR = xntr_pool.tile([P, NDT, QROWS], BF16)

                def xnt_sl(db, s):
                    # xn^T slice for global seq tile s, d-tile db
                    if s < NQT:
                        return XNT[:, db, s * P : (s + 1) * P]
                    return XNTR[:, db, (s - NQT) * P : (s - NQT + 1) * P]

                def load_w(eng, W_, w_d):
                    for _t in range(NDT):
                        eng.dma_start(
                            out=W_[:, _t, :],
                            in_=w_d.ap().rearrange("(t p) j -> p t j", p=P)[
                                :, _t, :
                            ],
                        )

                x_eng = [nc.sync, nc.scalar, nc.gpsimd]

                def ln_tile(s):
                    xt = x_pool.tile([P, D], F32, name="xt", tag="x")
                    x_eng[s % 3].dma_start(
                        out=xt, in_=x_d.ap()[s * P : (s + 1) * P, :]
                    )
                    st = stat_pool.tile([P, 2, 6], F32, name="st", tag="st")
                    nc.vector.bn_stats(out=st[:, 0], in_=xt[:, 0:512])
                    nc.vector.bn_stats(out=st[:, 1], in_=xt[:, 512:1024])
                    mv = stat_pool.tile([P, 2], F32, name="mv", tag="mv")
                    nc.vector.bn_aggr(out=mv, in_=st)
                    std = stat_pool.tile([P, 1], F32, name="sd", tag="sd")
                    nc.scalar.activation(
                        out=std, in_=mv[:, 1:2], func=AF.Sqrt, bias=eps_t
                    )
                    rstd = stat_pool.tile([P, 1], F32, name="rs", tag="rs")
                    nc.vector.reciprocal(out=rstd, in_=std)
                    xn = xn_pool.tile([P, D], BF16, name="xn", tag="xn")
                    nc.vector.tensor_scalar(
                        out=xn,
                        in0=xt,
                        scalar1=mv[:, 0:1],
                        scalar2=rstd,
                        op0=sub,
                        op1=mult,
                    )
                    for db in range(NDT):
                        ptr = psum_proj.tile([P, P], BF16, name="ptr", tag="tr")
                        nc.tensor.transpose(
                            ptr, xn[:, db * P : (db + 1) * P], ident
                        )
                        nc.vector.tensor_copy(xnt_sl(db, s), ptr)

                def v_proj(s):
                    for df in range(2):
                        ps = psum_proj.tile([P, 512], F32, name="ps", tag="pp")
                        for k in range(NDT):
                            nc.tensor.matmul(
                                ps,
                                lhsT=xnt_sl(k, s),
                                rhs=WV[:, k, df * 512 : (df + 1) * 512],
                                start=(k == 0),
                                stop=(k == NDT - 1),
                            )
                        ps_h = ps.rearrange("p (h e) -> p h e", e=HD)
                        nc.vector.tensor_copy(
                            Vr[:, s, df * 8 : (df + 1) * 8, 0:HD], ps_h
                        )

                def k_proj(kf):
                    # seq cols kf*512 .. (kf+1)*512 of K^T
                    for i in range(NDT):
                        ps = psum_proj.tile([P, 512], F32, name="ps", tag="pp")
                        for k in range(NDT):
                            nc.tensor.matmul(
                                ps,
                                lhsT=WK[:, k, i * P : (i + 1) * P],
                                rhs=(
                                    XNT[:, k, kf * 512 : (kf + 1) * 512]
                                    if kf < 2
                                    else XNTR[
                                        :, k, (kf - 2) * 512 : (kf - 1) * 512
                                    ]
                                ),
                                start=(k == 0),
                                stop=(k == NDT - 1),
                            )
                        nc.scalar.activation(
                            out=KT[:, i, kf * 512 : (kf + 1) * 512],
                            in_=ps,
                            func=AF.Copy,
                        )

                load_w(nc.scalar, WV, wvt_d)
                for s in range(4):
                    ln_tile(s)
                load_w(nc.gpsimd, WK, wkt_d)
                for s in range(4):
                    v_proj(s)
                for s in range(4, 8):
                    ln_tile(s)
                for s in range(4, 8):
                    v_proj(s)
                k_proj(0)
                k_proj(1)
                load_w(nc.gpsimd, WQ, wqt_d)
                for s in range(8, 12):
                    ln_tile(s)
                for s in range(8, 12):
                    v_proj(s)
                for s in range(12, 16):
                    ln_tile(s)
                for s in range(12, 16):
                    v_proj(s)
                k_proj(2)
                k_proj(3)
                # pair-0 Q projection here, so attention's first scores are
                # not gated by the attention-psum WAR on phase-1 banks
                for qf in range(2):
                    qp = psum_proj.tile([P, 512], F32, name="qp", tag="pp")
                    for k in range(NDT):
                        nc.tensor.matmul(
                            qp,
                            lhsT=WQ[:, k, 0:P],
                            rhs=XNT[:, k, qf * 512 : (qf + 1) * 512],
                            start=(k == 0),
                            stop=(k == NDT - 1),
                        )
                    nc.vector.tensor_scalar(
                        out=QT[:, 0, qf * 512 : (qf + 1) * 512],
                        in0=qp,
                        scalar1=bq_t[:, 0:1],
                        scalar2=None,
                        op0=add,
                    )

            # ---- phase 2: attention --------------------------------------
            with (
                tc.tile_pool(name="wo", bufs=1) as wo_pool,
                tc.tile_pool(name="crp", bufs=2) as cr_pool,
                tc.tile_pool(name="sep", bufs=2) as se_pool,
            ):
              WO = wo_pool.tile([P, NDT, D], BF16)
              for _t in range(NDT):
                  nc.sync.dma_start(
                      out=WO[:, _t, :],
                      in_=wot_d.ap().rearrange("(t p) j -> p t j", p=P)[:, _t, :],
                  )

              seb_cache = [None]

              def emit_normalize(ent, qfs=(0, 1), spread=True):
                  pt, cr = ent
                  if spread:
                      # spread the denominator row [1, 4*512] across 16
                      # partitions for the iterative-divide reciprocal
                      # (DVE reciprocal is ~8 cyc/elem/lane), via DRAM
                      nc.sync.dma_start(
                          out=recd_a[:],
                          in_=cr[HD : HD + 1, :, :].rearrange("p c q -> p (c q)"),
                      )
                      rs = se_pool.tile([16, 128], F32, name="rs", tag="rs")
                      nc.sync.dma_start(
                          out=rs,
                          in_=recd_a.rearrange("p (a b) -> (p a) b", a=16),
                      )
                      rr = se_pool.tile([16, 128], F32, name="rr", tag="rr")
                      nc.vector.reciprocal(out=rr, in_=rs)
                      nc.sync.dma_start(out=recd_b[0], in_=rr)
                      se0 = se_pool.tile(
                          [1, 4, 512], F32, name="se0", tag="se0", bufs=1
                      )
                      nc.sync.dma_start(
                          out=se0,
                          in_=recd_b.rearrange("p a b -> p (a b)").rearrange(
                              "p (c q) -> p c q", q=512
                          ),
                      )
                      seb_cache[0] = se0
                  se0 = seb_cache[0]
                  for qf in qfs:
                      for hi in range(2):
                          ch = hi * 2 + qf
                          seb = se_pool.tile([P, 512], F32, name="seb", tag="seb")
                          nc.gpsimd.partition_broadcast(seb[0:HD, :], se0[:, ch, :])
                          if hi == 0:
                              nc.vector.tensor_tensor(
                                  out=CT[0:HD, pt, qf * 512 : (qf + 1) * 512],
                                  in0=cr[0:HD, ch, :],
                                  in1=seb[0:HD, :],
                                  op=mult,
                              )
                          else:
                              tmp = se_pool.tile(
                                  [HD, 512], BF16, name="ctmp", tag="ctmp", bufs=1
                              )
                              nc.vector.tensor_tensor(
                                  out=tmp,
                                  in0=cr[0:HD, ch, :],
                                  in1=seb[0:HD, :],
                                  op=mult,
                              )
                              # partition shift 0..63 -> 64..127 via DMA
                              nc.gpsimd.dma_start(
                                  out=CT[HD:P, pt, qf * 512 : (qf + 1) * 512],
                                  in_=tmp,
                              )

              last_cr = [None]
              with (
                tc.tile_pool(name="probs", bufs=8) as probs_pool,
                tc.tile_pool(name="psum_sc", bufs=2, space="PSUM") as psum_sc,
                tc.tile_pool(name="psum_cx", bufs=4, space="PSUM") as psum_cx,
              ):
                def q_jit(tt):
                    qps = psum_sc.tile([P, QROWS], F32, name="qps", tag="s")
                    for qf in range(2):
                        for k in range(NDT):
                            nc.tensor.matmul(
                                qps[:, qf * 512 : (qf + 1) * 512],
                                lhsT=WQ[:, k, tt * P : (tt + 1) * P],
                                rhs=XNT[:, k, qf * 512 : (qf + 1) * 512],
                                start=(k == 0),
                                stop=(k == NDT - 1),
                            )
                    nc.vector.tensor_scalar(
                        out=QT[:, tt, :],
                        in0=qps,
                        scalar1=bq_t[:, tt : tt + 1],
                        scalar2=None,
                        op0=add,
                    )

                pending_norm = [None]
                for t in range(HP):
                    if pending_norm[0] is not None:
                        emit_normalize(pending_norm[0])
                        pending_norm[0] = None

                    probs = [[None] * NST for _ in range(2)]
                    cx = [[None] * 2 for _ in range(2)]  # [hi][qf]

                    def emit_scores(kt):
                        kl = KT[:, t, kt * P : (kt + 1) * P]
                        for hi in range(2):
                            off = hi * HD
                            sps = psum_sc.tile([P, QROWS], F32, name="sps", tag="s")
                            for qf in range(2):
                                nc.tensor.matmul(
                                    sps[:, qf * 512 : (qf + 1) * 512],
                                    lhsT=kl[off : off + HD, :],
                                    rhs=QT[off : off + HD, t, qf * 512 : (qf + 1) * 512],
                                    start=True,
                                    stop=True,
                                    tile_position=(off, 0),
                                )
                            pt = probs_pool.tile([P, QROWS], BF16, name="pt", tag="p")
                            nc.scalar.activation(out=pt, in_=sps, func=AF.Exp)
                            probs[hi][kt] = pt

                    def emit_ctx(kt):
                        for hi in range(2):
                            h = 2 * t + hi
                            for qf in range(2):
                                if kt == 0:
                                    cx[hi][qf] = psum_cx.tile(
                                        [P, 512], F32, name="cx", tag="cx"
                                    )
                                nc.tensor.matmul(
                                    cx[hi][qf][0:VSTRIDE, :],
                                    lhsT=Vr[:, kt, h, :],
                                    rhs=probs[hi][kt][:, qf * 512 : (qf + 1) * 512],
                                    start=(kt == 0),
                                    stop=(kt == NST - 1),
                                )

                    for kt in range(NST):
                        emit_scores(kt)
                        if kt == NST - 2 and t + 1 < HP:
                            q_jit(t + 1)
                        if kt >= 1:
                            emit_ctx(kt - 1)
                    emit_ctx(NST - 1)

                    # evacuate ctx chains to SBUF fast so the PSUM banks free
                    # up for the next pair; rows 0..63 = unnormalized ctx,
                    # row 64 = sum(exp)
                    cr = cr_pool.tile([VSTRIDE, 4, 512], F32, name="cr", tag="cr")
                    for hi in range(2):
                        for qf in range(2):
                            nc.vector.tensor_copy(
                                cr[:, hi * 2 + qf, :], cx[hi][qf][0:VSTRIDE, :]
                            )
                    if t == HP - 1:
                        last_cr[0] = (t, cr)
                    else:
                        pending_norm[0] = (t, cr)

              # ---- final normalize + output projection, qf-interleaved ----
              with (
                  tc.tile_pool(name="osb", bufs=3) as osb_pool,
                  tc.tile_pool(name="psum_o", bufs=8, space="PSUM") as psum_o,
              ):
                  def out_proj_half(qts):
                      # accumulate pairs 0..6 for all chains first; the pair-7
                      # contribution lands after its normalize completes
                      chains = {}
                      for qt in qts:
                          for jf in range(2):
                              ps = psum_o.tile([P, 512], F32, name="ps", tag="po")
                              chains[(qt, jf)] = ps
                              for i in range(NDT - 1):
                                  nc.tensor.matmul(
                                      ps,
                                      lhsT=CT[:, i, qt * P : (qt + 1) * P],
                                      rhs=WO[:, i, jf * 512 : (jf + 1) * 512],
                                      start=(i == 0),
                                      stop=False,
                                  )
                      for qt in qts:
                          ot = osb_pool.tile([P, D], F32, name="ot", tag="o")
                          for jf in range(2):
                              ps = chains[(qt, jf)]
                              nc.tensor.matmul(
                                  ps,
                                  lhsT=CT[:, NDT - 1, qt * P : (qt + 1) * P],
                                  rhs=WO[:, NDT - 1, jf * 512 : (jf + 1) * 512],
                                  start=False,
                                  stop=True,
                              )
                              nc.vector.tensor_tensor(
                                  out=ot[:, jf * 512 : (jf + 1) * 512],
                                  in0=ps,
                                  in1=bob[:, jf * 512 : (jf + 1) * 512],
                                  op=add,
                              )
                          nc.sync.dma_start(
                              out=out_d.ap()[qt * P : (qt + 1) * P, :], in_=ot
                          )

                  emit_normalize(last_cr[0], qfs=(0,))
                  out_proj_half(range(4))
                  emit_normalize(last_cr[0], qfs=(1,), spread=False)
                  out_proj_half(range(4, NQT))

    nc.compile()
    return nc


_NC_CACHE = None


def _get_program():
    global _NC_CACHE
    if _NC_CACHE is None:
        _NC_CACHE = build_program()
    return _NC_CACHE


def _prep_host(x, ln_gamma, ln_beta, Wq, bq, Wk, bk, Wv, bv, Wo, bo):
    bf16 = ml_dtypes.bfloat16
    g = np.asarray(ln_gamma, np.float64)
    be = np.asarray(ln_beta, np.float64)
    scale = 1.0 / np.sqrt(np.float64(HD))

    def fold(W, b, s=1.0):
        W = np.asarray(W, np.float64)
        b = np.asarray(b, np.float64)
        W_eff = W * g[None, :] * s
        b_eff = (b + W @ be) * s
        wt = np.ascontiguousarray(W_eff.T).astype(bf16)
        return wt, b_eff.astype(np.float32)

    wqt, bq_e = fold(Wq, bq, scale)
    wkt, _ = fold(Wk, bk)           # K bias cancels in softmax
    wvt, bv_e = fold(Wv, bv)
    Wo64 = np.asarray(Wo, np.float64)
    wot = np.ascontiguousarray(Wo64.T).astype(bf16)
    # ctx rows carry +bv_eff (per-head value bias); fold it through Wo into bo
    bo_e = (np.asarray(bo, np.float64) + Wo64 @ np.asarray(bv_e, np.float64)
            ).astype(np.float32)

    shared = {
        "wqt": wqt,
        "wkt": wkt,
        "wvt": wvt,
        "wot": wot,
        "bq": bq_e.reshape(NDT, P),
        "bo": bo_e.reshape(1, D),
        "ident": np.eye(P, dtype=bf16),
    }
    x = np.asarray(x, np.float32)
    in_maps = []
    for c in range(NCORES):
        b_idx, half = c // 2, c % 2
        # local query half first, partner half second
        x_local = np.concatenate(
            [
                x[b_idx, half * QROWS : (half + 1) * QROWS],
                x[b_idx, (1 - half) * QROWS : (2 - half) * QROWS],
            ]
        )
        in_maps.append({"x": np.ascontiguousarray(x_local), **shared})
    return in_maps


def kernel(x, ln_gamma, ln_beta, Wq, bq, Wk, bk, Wv, bv, Wo, bo):
    nc = _get_program()
    in_maps = _prep_host(x, ln_gamma, ln_beta, Wq, bq, Wk, bk, Wv, bv, Wo, bo)
    res = run_bass_kernel_spmd(nc, in_maps, core_ids=list(range(NCORES)))
    out = np.empty((B, S, D), np.float32)
    for c in range(NCORES):
        b_idx, half = c // 2, c % 2
        out[b_idx, half * QROWS : (half + 1) * QROWS] = res.results[c]["out"]
    return out


if __name__ == "__main__":
    build_program()
    print("program built OK")
